# revision 19
# baseline (speedup 1.0000x reference)
"""Trainium2 Bass kernel for nn_Aggregate (segment_reduce).

Computes out[b, g] = sum_{c : segment_ids[c] == g} x[b, c] for
x: [8192, 8192] f32, segment_ids: [8192] int32 (values in [0, 512)),
out: [8192, 512] f32.

Strategy (8 NeuronCores, data-parallel over the batch dim, no collectives):
  - Each core gets a 1024-row shard of x and computes its shard of out
    independently.
  - The column->group reduction is out = x @ M with M the [8192, 512]
    one-hot segment matrix.  M is built ON DEVICE (64 DVE `is_equal` ops
    against an iota row: M[c, g] = (seg[c] == g)) so the only meaningful
    HBM traffic is x itself.
  - Per 128-row block: DMA the block in, transpose each [128, 128] column
    chunk on the TensorEngine (float32r transpose, 1.5 cyc/row), evacuate
    transposed chunks from PSUM to SBUF (DVE/ACT alternating), then run 64
    accumulating float32r matmuls (1 cycle/row) into one PSUM bank:
    psum[b, g] += xT_k.T @ M_k.
  - float32r rounds x to ~12 mantissa bits on PE ingest; the one-hot side
    is exact and PSUM accumulates in fp32, so the absmax-relative error is
    ~1e-4 (residual variance ~1e-8).
"""

import sys

sys.path.insert(0, "/opt/trn_rl_repo")

import numpy as np

import concourse.bass as bass
import concourse.tile as tile
from concourse import mybir
from concourse.bass_utils import run_bass_kernel_spmd

BATCH = 8192
C = 8192
G = 512
N_CORES = 8
B_SHARD = BATCH // N_CORES  # 1024 rows per core
N_BLK = B_SHARD // 128      # 8 blocks of 128 rows
N_CH = C // 128             # 64 column chunks
F32 = mybir.dt.float32
F32R = mybir.dt.float32r


def _split_multiwaits(nc):
    """The walrus build here accepts only one sync-wait per instruction.
    Hoist extra waits onto InstNoOp instructions inserted right before the
    owner on the same engine (the sequencer executes waits in order, so
    semantics are unchanged)."""
    n_new = 0
    for f in nc.m.functions:
        for bb in f.blocks:
            new_insts = []
            for inst in bb.instructions:
                si = inst.sync_info
                if si is not None and si.on_wait and len(si.on_wait) > 1:
                    waits = list(si.on_wait)
                    for w in waits[:-1]:
                        nop = mybir.InstNoOp(
                            name=f"I-waitsplit-{n_new}", ins=[], outs=[]
                        )
                        nop.engine = inst.engine
                        nop.sync_info = mybir.SyncInfo(on_wait=[w], on_update=[])
                        new_insts.append(nop)
                        n_new += 1
                    si.on_wait = [waits[-1]]
                new_insts.append(inst)
            bb.instructions[:] = new_insts
    return n_new


def _build_nc():
    nc = bass.Bass(
        "TRN2", target_bir_lowering=False, debug=False, num_devices=N_CORES
    )
    x_d = nc.dram_tensor("x", [B_SHARD, C], F32R, kind="ExternalInput").ap()
    # seg values as f32, laid out [128 c-local, 64 chunk]
    seg_d = nc.dram_tensor("seg", [128, N_CH], F32, kind="ExternalInput").ap()
    # iota row 0..511 replicated across 128 partitions
    iota_d = nc.dram_tensor("iota", [128, G], F32, kind="ExternalInput").ap()
    id_d = nc.dram_tensor("ident", [128, 128], F32R, kind="ExternalInput").ap()
    out_d = nc.dram_tensor("out", [B_SHARD, G], F32, kind="ExternalOutput").ap()

    with tile.TileContext(nc) as tc:
        with tc.tile_pool(name="const", bufs=1) as cpool, \
             tc.tile_pool(name="xp", bufs=3) as xpool, \
             tc.tile_pool(name="xt", bufs=10) as xtp, \
             tc.tile_pool(name="so", bufs=2) as sop, \
             tc.tile_pool(name="trp", bufs=3, space="PSUM") as trpp, \
             tc.tile_pool(name="acc", bufs=2, space="PSUM") as accp:
            # First x piece before the consts so the PE starts ASAP.
            xp0 = xpool.tile([128, 1024], F32R, tag="x")
            nc.sync.dma_start(xp0[:], x_d[0:128, 0:1024])
            ident = cpool.tile([128, 128], F32R, tag="id")
            nc.sync.dma_start(ident[:], id_d[:])
            segt = cpool.tile([128, N_CH], F32, tag="seg")
            nc.sync.dma_start(segt[:], seg_d[:])
            iot = cpool.tile([128, G], F32, tag="iota")
            nc.sync.dma_start(iot[:], iota_d[:])

            # One-hot M built on device: 128KB/partition, float32r.
            # (DVE only: this walrus rejects TensorTensor on Pool.)
            mt = cpool.tile([128, N_CH * G], F32R, tag="m")

            def build_m_batch(j, half):
                # chunks in one DVE op via stride-0 broadcast APs:
                # mt[p, k, g] = (segt[p, k] == iota[p, g])
                lo, n = 8 * j + 4 * half, 4
                out_v = mt[:, G * lo:G * (lo + n)].rearrange(
                    "p (k g) -> p k g", g=G
                )
                seg_s = segt[:, lo:lo + n]
                seg_v = bass.AP(
                    tensor=seg_s.tensor, offset=seg_s.offset,
                    ap=[seg_s.ap[0], seg_s.ap[1], [0, G]],
                )
                iot_s = iot[:]
                iot_v = bass.AP(
                    tensor=iot_s.tensor, offset=iot_s.offset,
                    ap=[iot_s.ap[0], [0, n], iot_s.ap[1]],
                )
                nc.vector.tensor_tensor(
                    out_v, seg_v, iot_v, op=mybir.AluOpType.is_equal
                )

            for blk in range(N_BLK):
                xts = []
                for j in range(N_CH // 8):  # 8 batches of 8 chunks
                    if blk == 0:
                        build_m_batch(j, 0)
                        build_m_batch(j, 1)
                    # one 512KB x piece per batch: transposes start as soon
                    # as the first piece lands, and pieces free up quickly.
                    if blk == 0 and j == 0:
                        xp = xp0
                    else:
                        xp = xpool.tile([128, 1024], F32R, tag="x")
                        nc.sync.dma_start(
                            xp[:],
                            x_d[blk * 128:(blk + 1) * 128,
                                1024 * j:1024 * (j + 1)],
                        )
                    trp = trpp.tile([128, 1024], F32R, tag="trp")
                    for s in range(8):
                        nc.tensor.transpose(
                            trp[:, 128 * s:128 * (s + 1)],
                            xp[:, 128 * s:128 * (s + 1)],
                            ident[:],
                        )
                    xt = xtp.tile([128, 1024], F32R, tag="xt")
                    # ACT-heavy split: ACT's per-copy cost is lower and DVE
                    # also carries the M build (blocks 0-1: all on ACT).
                    if blk > 1 and j % 8 in (0, 3, 6):
                        nc.vector.tensor_copy(xt[:], trp[:])
                    else:
                        nc.scalar.copy(xt[:], trp[:])
                    xts.append(xt)

                acc = accp.tile([128, G], F32, tag="acc")
                for k in range(N_CH):
                    xt_v = xts[k // 8][:, 128 * (k % 8):128 * (k % 8 + 1)]
                    nc.tensor.matmul(
                        acc[:], xt_v, mt[:, G * k:G * (k + 1)],
                        start=(k == 0), stop=(k == N_CH - 1),
                        skip_group_check=True,
                    )
                so = sop.tile([128, G], F32, tag="so")
                if blk <= 1:
                    nc.scalar.copy(so[:], acc[:])
                else:
                    nc.vector.tensor_copy(so[:], acc[:])
                nc.sync.dma_start(out_d[blk * 128:(blk + 1) * 128, :], so[:])

    _split_multiwaits(nc)
    return nc


_NC_CACHE = {}


def _get_nc():
    if "nc" not in _NC_CACHE:
        _NC_CACHE["nc"] = _build_nc()
    return _NC_CACHE["nc"]


def kernel(x: np.ndarray, segment_ids: np.ndarray) -> np.ndarray:
    x = np.ascontiguousarray(x, dtype=np.float32)
    assert x.shape == (BATCH, C)
    seg = np.asarray(segment_ids).astype(np.int64).ravel()
    assert seg.shape == (C,)
    nc = _get_nc()
    seg_np = seg.reshape(N_CH, 128).T.astype(np.float32)  # [128 c-local, 64]
    seg_np = np.ascontiguousarray(seg_np)
    iota_np = np.broadcast_to(
        np.arange(G, dtype=np.float32), (128, G)
    ).copy()
    ident = np.eye(128, dtype=np.float32)
    ins = [
        {
            "x": x[i * B_SHARD:(i + 1) * B_SHARD],
            "seg": seg_np,
            "iota": iota_np,
            "ident": ident,
        }
        for i in range(N_CORES)
    ]
    res = run_bass_kernel_spmd(nc, ins, core_ids=list(range(N_CORES)))
    out = np.concatenate(
        [res.results[i]["out"] for i in range(N_CORES)], axis=0
    )
    return np.ascontiguousarray(out, dtype=np.float32)


if __name__ == "__main__":
    rng = np.random.default_rng(0)
    x = rng.standard_normal((BATCH, C), dtype=np.float32)
    seg = rng.integers(0, G, C).astype(np.int32)
    out = kernel(x, seg)
    onehot = np.zeros((C, G), np.float64)
    onehot[np.arange(C), seg] = 1.0
    exp = x.astype(np.float64) @ onehot
    err = np.abs(out - exp).max() / np.abs(exp).max()
    print("selftest absmax-rel err:", err)


# revision 25
# speedup vs baseline: 1.0004x; 1.0004x over previous
"""Trainium2 Bass kernel for nn_Aggregate (segment_reduce).

Computes out[b, g] = sum_{c : segment_ids[c] == g} x[b, c] for
x: [8192, 8192] f32, segment_ids: [8192] int32 (values in [0, 512)),
out: [8192, 512] f32.

Strategy (8 NeuronCores, data-parallel over the batch dim, no collectives):
  - Each core gets a 1024-row shard of x and computes its shard of out
    independently.
  - The column->group reduction is out = x @ M with M the [8192, 512]
    one-hot segment matrix.  M is built ON DEVICE (64 DVE `is_equal` ops
    against an iota row: M[c, g] = (seg[c] == g)) so the only meaningful
    HBM traffic is x itself.
  - Per 128-row block: DMA the block in, transpose each [128, 128] column
    chunk on the TensorEngine (float32r transpose, 1.5 cyc/row), evacuate
    transposed chunks from PSUM to SBUF (DVE/ACT alternating), then run 64
    accumulating float32r matmuls (1 cycle/row) into one PSUM bank:
    psum[b, g] += xT_k.T @ M_k.
  - float32r rounds x to ~12 mantissa bits on PE ingest; the one-hot side
    is exact and PSUM accumulates in fp32, so the absmax-relative error is
    ~1e-4 (residual variance ~1e-8).
"""

import sys

sys.path.insert(0, "/opt/trn_rl_repo")

import numpy as np

import concourse.bass as bass
import concourse.tile as tile
from concourse import mybir
from concourse.bass_utils import run_bass_kernel_spmd

BATCH = 8192
C = 8192
G = 512
N_CORES = 8
B_SHARD = BATCH // N_CORES  # 1024 rows per core
N_BLK = B_SHARD // 128      # 8 blocks of 128 rows
N_CH = C // 128             # 64 column chunks
F32 = mybir.dt.float32
F32R = mybir.dt.float32r


def _split_multiwaits(nc):
    """The walrus build here accepts only one sync-wait per instruction.
    Hoist extra waits onto InstNoOp instructions inserted right before the
    owner on the same engine (the sequencer executes waits in order, so
    semantics are unchanged)."""
    n_new = 0
    for f in nc.m.functions:
        for bb in f.blocks:
            new_insts = []
            for inst in bb.instructions:
                si = inst.sync_info
                if si is not None and si.on_wait and len(si.on_wait) > 1:
                    waits = list(si.on_wait)
                    for w in waits[:-1]:
                        nop = mybir.InstNoOp(
                            name=f"I-waitsplit-{n_new}", ins=[], outs=[]
                        )
                        nop.engine = inst.engine
                        nop.sync_info = mybir.SyncInfo(on_wait=[w], on_update=[])
                        new_insts.append(nop)
                        n_new += 1
                    si.on_wait = [waits[-1]]
                new_insts.append(inst)
            bb.instructions[:] = new_insts
    return n_new


def _build_nc():
    nc = bass.Bass(
        "TRN2", target_bir_lowering=False, debug=False, num_devices=N_CORES
    )
    x_d = nc.dram_tensor("x", [B_SHARD, C], F32R, kind="ExternalInput").ap()
    # seg values as f32, laid out [128 c-local, 64 chunk]
    seg_d = nc.dram_tensor("seg", [128, N_CH], F32, kind="ExternalInput").ap()
    # iota row 0..511 replicated across 128 partitions
    iota_d = nc.dram_tensor("iota", [128, G], F32, kind="ExternalInput").ap()
    id_d = nc.dram_tensor("ident", [128, 128], F32R, kind="ExternalInput").ap()
    out_d = nc.dram_tensor("out", [B_SHARD, G], F32, kind="ExternalOutput").ap()

    with tile.TileContext(nc) as tc:
        with tc.tile_pool(name="const", bufs=1) as cpool, \
             tc.tile_pool(name="xp", bufs=3) as xpool, \
             tc.tile_pool(name="xt", bufs=10) as xtp, \
             tc.tile_pool(name="so", bufs=3) as sop, \
             tc.tile_pool(name="trp", bufs=3, space="PSUM") as trpp, \
             tc.tile_pool(name="acc", bufs=2, space="PSUM") as accp:
            # First x piece before the consts so the PE starts ASAP.
            xp0 = xpool.tile([128, 1024], F32R, tag="x")
            nc.sync.dma_start(xp0[:], x_d[0:128, 0:1024])
            ident = cpool.tile([128, 128], F32R, tag="id")
            nc.sync.dma_start(ident[:], id_d[:])
            segt = cpool.tile([128, N_CH], F32, tag="seg")
            nc.sync.dma_start(segt[:], seg_d[:])
            iot = cpool.tile([128, G], F32, tag="iota")
            nc.sync.dma_start(iot[:], iota_d[:])

            # One-hot M built on device: 128KB/partition, float32r.
            # (DVE only: this walrus rejects TensorTensor on Pool.)
            mt = cpool.tile([128, N_CH * G], F32R, tag="m")

            def build_m_batch(j, half):
                # chunks in one DVE op via stride-0 broadcast APs:
                # mt[p, k, g] = (segt[p, k] == iota[p, g])
                lo, n = 8 * j + 4 * half, 4
                out_v = mt[:, G * lo:G * (lo + n)].rearrange(
                    "p (k g) -> p k g", g=G
                )
                seg_s = segt[:, lo:lo + n]
                seg_v = bass.AP(
                    tensor=seg_s.tensor, offset=seg_s.offset,
                    ap=[seg_s.ap[0], seg_s.ap[1], [0, G]],
                )
                iot_s = iot[:]
                iot_v = bass.AP(
                    tensor=iot_s.tensor, offset=iot_s.offset,
                    ap=[iot_s.ap[0], [0, n], iot_s.ap[1]],
                )
                nc.vector.tensor_tensor(
                    out_v, seg_v, iot_v, op=mybir.AluOpType.is_equal
                )

            for blk in range(N_BLK):
                xts = []
                for j in range(N_CH // 8):  # 8 batches of 8 chunks
                    if blk == 0:
                        build_m_batch(j, 0)
                        build_m_batch(j, 1)
                    # one 512KB x piece per batch: transposes start as soon
                    # as the first piece lands, and pieces free up quickly.
                    if blk == 0 and j == 0:
                        xp = xp0
                    else:
                        xp = xpool.tile([128, 1024], F32R, tag="x")
                        nc.sync.dma_start(
                            xp[:],
                            x_d[blk * 128:(blk + 1) * 128,
                                1024 * j:1024 * (j + 1)],
                        )
                    trp = trpp.tile([128, 1024], F32R, tag="trp")
                    for s in range(8):
                        nc.tensor.transpose(
                            trp[:, 128 * s:128 * (s + 1)],
                            xp[:, 128 * s:128 * (s + 1)],
                            ident[:],
                        )
                    xt = xtp.tile([128, 1024], F32R, tag="xt")
                    # ACT-heavy split: ACT's per-copy cost is lower and DVE
                    # also carries the M build (blocks 0-1: all on ACT).
                    if blk > 1 and j % 8 in (0, 3, 6):
                        nc.vector.tensor_copy(xt[:], trp[:])
                    else:
                        nc.scalar.copy(xt[:], trp[:])
                    xts.append(xt)

                acc = accp.tile([128, G], F32, tag="acc")
                for k in range(N_CH):
                    xt_v = xts[k // 8][:, 128 * (k % 8):128 * (k % 8 + 1)]
                    nc.tensor.matmul(
                        acc[:], xt_v, mt[:, G * k:G * (k + 1)],
                        start=(k == 0), stop=(k == N_CH - 1),
                        skip_group_check=True,
                    )
                so = sop.tile([128, G], F32, tag="so")
                if blk <= 1:
                    nc.scalar.copy(so[:], acc[:])
                else:
                    nc.vector.tensor_copy(so[:], acc[:])
                nc.sync.dma_start(out_d[blk * 128:(blk + 1) * 128, :], so[:])

    _split_multiwaits(nc)
    return nc


_NC_CACHE = {}


def _get_nc():
    if "nc" not in _NC_CACHE:
        _NC_CACHE["nc"] = _build_nc()
    return _NC_CACHE["nc"]


def kernel(x: np.ndarray, segment_ids: np.ndarray) -> np.ndarray:
    x = np.ascontiguousarray(x, dtype=np.float32)
    assert x.shape == (BATCH, C)
    seg = np.asarray(segment_ids).astype(np.int64).ravel()
    assert seg.shape == (C,)
    nc = _get_nc()
    seg_np = seg.reshape(N_CH, 128).T.astype(np.float32)  # [128 c-local, 64]
    seg_np = np.ascontiguousarray(seg_np)
    iota_np = np.broadcast_to(
        np.arange(G, dtype=np.float32), (128, G)
    ).copy()
    ident = np.eye(128, dtype=np.float32)
    ins = [
        {
            "x": x[i * B_SHARD:(i + 1) * B_SHARD],
            "seg": seg_np,
            "iota": iota_np,
            "ident": ident,
        }
        for i in range(N_CORES)
    ]
    res = run_bass_kernel_spmd(nc, ins, core_ids=list(range(N_CORES)))
    out = np.concatenate(
        [res.results[i]["out"] for i in range(N_CORES)], axis=0
    )
    return np.ascontiguousarray(out, dtype=np.float32)


if __name__ == "__main__":
    rng = np.random.default_rng(0)
    x = rng.standard_normal((BATCH, C), dtype=np.float32)
    seg = rng.integers(0, G, C).astype(np.int32)
    out = kernel(x, seg)
    onehot = np.zeros((C, G), np.float64)
    onehot[np.arange(C), seg] = 1.0
    exp = x.astype(np.float64) @ onehot
    err = np.abs(out - exp).max() / np.abs(exp).max()
    print("selftest absmax-rel err:", err)


# revision 29
# speedup vs baseline: 1.1135x; 1.1131x over previous
"""Trainium2 Bass kernel for nn_Aggregate (segment_reduce).

Computes out[b, g] = sum_{c : segment_ids[c] == g} x[b, c] for
x: [8192, 8192] f32, segment_ids: [8192] int32 (values in [0, 512)),
out: [8192, 512] f32.

Strategy (8 NeuronCores, data-parallel over the batch dim, no collectives):
  - Each core gets a 1024-row shard of x and computes its shard of out
    independently.
  - The column->group reduction is out = x @ M with M the [8192, 512]
    one-hot segment matrix, built ON DEVICE from segment_ids (DVE
    `is_equal` against an iota row at 2x fp16 rate), so the only
    meaningful HBM traffic is x itself.
  - x is converted fp32->fp16 inline by the SWDGE cast-DMA during the
    load (HBM reads stay fp32; the conversion is free in the SDMA
    datapath and bit-exact vs a host fp16 cast).
  - Per 128-row block: 8x 512KB piece DMAs, 64 TensorEngine fp16
    transposes (1 cyc/row) into PSUM, DVE/ACT evacuation to fp16 SBUF,
    then 64 accumulating fp16 matmuls (1 cyc/row) into one fp32 PSUM
    bank: psum[b, g] += xT_k.T @ M_k.
  - fp16 rounds x to 11 mantissa bits; the one-hot side is exact and
    PSUM accumulates in fp32, so the absmax-relative error is ~2.2e-4
    (residual variance ~4e-8).  kernel_f32r.py is the float32r variant
    (~11% slower, ~1.1e-4 error) if a tighter tolerance is ever needed.
"""

import sys

sys.path.insert(0, "/opt/trn_rl_repo")

import numpy as np

import concourse.bass as bass
import concourse.tile as tile
from concourse import mybir
from concourse.bass_utils import run_bass_kernel_spmd

BATCH = 8192
C = 8192
G = 512
N_CORES = 8
B_SHARD = BATCH // N_CORES  # 1024 rows per core
N_BLK = B_SHARD // 128      # 8 blocks of 128 rows
N_CH = C // 128             # 64 column chunks
F32 = mybir.dt.float32
F32R = mybir.dt.float32r
F16 = mybir.dt.float16


def _split_multiwaits(nc):
    """The walrus build here accepts only one sync-wait per instruction.
    Hoist extra waits onto InstNoOp instructions inserted right before the
    owner on the same engine (the sequencer executes waits in order, so
    semantics are unchanged)."""
    n_new = 0
    for f in nc.m.functions:
        for bb in f.blocks:
            new_insts = []
            for inst in bb.instructions:
                si = inst.sync_info
                if si is not None and si.on_wait and len(si.on_wait) > 1:
                    waits = list(si.on_wait)
                    for w in waits[:-1]:
                        nop = mybir.InstNoOp(
                            name=f"I-waitsplit-{n_new}", ins=[], outs=[]
                        )
                        nop.engine = inst.engine
                        nop.sync_info = mybir.SyncInfo(on_wait=[w], on_update=[])
                        new_insts.append(nop)
                        n_new += 1
                    si.on_wait = [waits[-1]]
                new_insts.append(inst)
            bb.instructions[:] = new_insts
    return n_new


def _build_nc():
    nc = bass.Bass(
        "TRN2", target_bir_lowering=False, debug=False, num_devices=N_CORES
    )
    x_d = nc.dram_tensor("x", [B_SHARD, C], F32, kind="ExternalInput").ap()
    # seg values as f32, laid out [128 c-local, 64 chunk]
    seg_d = nc.dram_tensor("seg", [128, N_CH], F16, kind="ExternalInput").ap()
    # iota row 0..511 replicated across 128 partitions
    iota_d = nc.dram_tensor("iota", [128, G], F16, kind="ExternalInput").ap()
    id_d = nc.dram_tensor("ident", [128, 128], F16, kind="ExternalInput").ap()
    out_d = nc.dram_tensor("out", [B_SHARD, G], F32, kind="ExternalOutput").ap()

    with tile.TileContext(nc) as tc:
        with tc.tile_pool(name="const", bufs=1) as cpool, \
             tc.tile_pool(name="xp", bufs=10) as xpool, \
             tc.tile_pool(name="xt", bufs=16) as xtp, \
             tc.tile_pool(name="so", bufs=2) as sop, \
             tc.tile_pool(name="trp", bufs=5, space="PSUM") as trpp, \
             tc.tile_pool(name="acc", bufs=2, space="PSUM") as accp:
            # First x piece before the consts so the PE starts ASAP.
            xp0 = xpool.tile([128, 1024], F16, tag="x")
            nc.gpsimd.dma_start(xp0[:], x_d[0:128, 0:1024])
            ident = cpool.tile([128, 128], F16, tag="id")
            nc.sync.dma_start(ident[:], id_d[:])
            segt = cpool.tile([128, N_CH], F16, tag="seg")
            nc.sync.dma_start(segt[:], seg_d[:])
            iot = cpool.tile([128, G], F16, tag="iota")
            nc.sync.dma_start(iot[:], iota_d[:])

            # One-hot M built on device: 128KB/partition, float32r.
            # (DVE only: this walrus rejects TensorTensor on Pool.)
            mt = cpool.tile([128, N_CH * G], F16, tag="m")

            def build_m_batch(j, half):
                # chunks in one DVE op via stride-0 broadcast APs:
                # mt[p, k, g] = (segt[p, k] == iota[p, g])
                lo, n = 8 * j + 4 * half, 4
                out_v = mt[:, G * lo:G * (lo + n)].rearrange(
                    "p (k g) -> p k g", g=G
                )
                seg_s = segt[:, lo:lo + n]
                seg_v = bass.AP(
                    tensor=seg_s.tensor, offset=seg_s.offset,
                    ap=[seg_s.ap[0], seg_s.ap[1], [0, G]],
                )
                iot_s = iot[:]
                iot_v = bass.AP(
                    tensor=iot_s.tensor, offset=iot_s.offset,
                    ap=[iot_s.ap[0], [0, n], iot_s.ap[1]],
                )
                nc.vector.tensor_tensor(
                    out_v, seg_v, iot_v, op=mybir.AluOpType.is_equal
                )

            for blk in range(N_BLK):
                xts = []
                for j in range(N_CH // 8):  # 8 batches of 8 chunks
                    if blk == 0:
                        build_m_batch(j, 0)
                        build_m_batch(j, 1)
                    # one 512KB x piece per batch: transposes start as soon
                    # as the first piece lands, and pieces free up quickly.
                    if blk == 0 and j == 0:
                        xp = xp0
                    else:
                        xp = xpool.tile([128, 1024], F16, tag="x")
                        nc.gpsimd.dma_start(
                            xp[:],
                            x_d[blk * 128:(blk + 1) * 128,
                                1024 * j:1024 * (j + 1)],
                        )
                    trp = trpp.tile([128, 1024], F16, tag="trp")
                    for s in range(8):
                        nc.tensor.transpose(
                            trp[:, 128 * s:128 * (s + 1)],
                            xp[:, 128 * s:128 * (s + 1)],
                            ident[:],
                        )
                    xt = xtp.tile([128, 1024], F16, tag="xt")
                    # ACT-heavy split: ACT's per-copy cost is lower and DVE
                    # also carries the M build (blocks 0-1: all on ACT).
                    if blk > 1 and j % 8 in (0, 3, 6):
                        nc.vector.tensor_copy(xt[:], trp[:])
                    else:
                        nc.scalar.copy(xt[:], trp[:])
                    xts.append(xt)

                acc = accp.tile([128, G], F32, tag="acc")
                for k in range(N_CH):
                    xt_v = xts[k // 8][:, 128 * (k % 8):128 * (k % 8 + 1)]
                    nc.tensor.matmul(
                        acc[:], xt_v, mt[:, G * k:G * (k + 1)],
                        start=(k == 0), stop=(k == N_CH - 1),
                        skip_group_check=True,
                    )
                so = sop.tile([128, G], F32, tag="so")
                if blk <= 1:
                    nc.scalar.copy(so[:], acc[:])
                else:
                    nc.vector.tensor_copy(so[:], acc[:])
                nc.sync.dma_start(out_d[blk * 128:(blk + 1) * 128, :], so[:])

    _split_multiwaits(nc)
    return nc


_NC_CACHE = {}


def _get_nc():
    if "nc" not in _NC_CACHE:
        _NC_CACHE["nc"] = _build_nc()
    return _NC_CACHE["nc"]


def kernel(x: np.ndarray, segment_ids: np.ndarray) -> np.ndarray:
    x = np.ascontiguousarray(x, dtype=np.float32)
    assert x.shape == (BATCH, C)
    seg = np.asarray(segment_ids).astype(np.int64).ravel()
    assert seg.shape == (C,)
    nc = _get_nc()
    seg_np = seg.reshape(N_CH, 128).T.astype(np.float16)  # [128 c-local, 64]
    seg_np = np.ascontiguousarray(seg_np)
    iota_np = np.broadcast_to(
        np.arange(G, dtype=np.float16), (128, G)
    ).copy()
    ident = np.eye(128, dtype=np.float16)
    ins = [
        {
            "x": x[i * B_SHARD:(i + 1) * B_SHARD],
            "seg": seg_np,
            "iota": iota_np,
            "ident": ident,
        }
        for i in range(N_CORES)
    ]
    res = run_bass_kernel_spmd(nc, ins, core_ids=list(range(N_CORES)))
    out = np.concatenate(
        [res.results[i]["out"] for i in range(N_CORES)], axis=0
    )
    return np.ascontiguousarray(out, dtype=np.float32)


if __name__ == "__main__":
    rng = np.random.default_rng(0)
    x = rng.standard_normal((BATCH, C), dtype=np.float32)
    seg = rng.integers(0, G, C).astype(np.int32)
    out = kernel(x, seg)
    onehot = np.zeros((C, G), np.float64)
    onehot[np.arange(C), seg] = 1.0
    exp = x.astype(np.float64) @ onehot
    err = np.abs(out - exp).max() / np.abs(exp).max()
    print("selftest absmax-rel err:", err)
